# revision 34
# baseline (speedup 1.0000x reference)
"""BatchHardTripletLoss on 8 Trainium2 NeuronCores — flipped + norm-dealt.

Layout: rows label-sorted on host; each core streams its 1024 anchors
(free dim) against all B=8192 embeddings as 64 column chunks of 128
(partition dim), processed as 32 psum pairs [128, 2048].

  - Own pairs (K/2 ~ 5): chunks holding the core's own labels (plus
    fillers), spaced through the schedule. Raw -2x.x tiles are ACT-copied
    to f16 and shipped to the HOST over the idle DMA path; the host adds
    exact sq_j, masks same-label/self pairs, and computes hardest-pos
    plus the own-side hardest-neg. No penalty matmuls, no own DVE work.
  - Dealt pairs: remaining columns norm-sorted and dealt so sq_j is
    near-constant per partition. Two paths balanced across engines:
    (A) ACT Copy + one wide [2048] f16 chain min on DVE; (B) one direct
    chain min against the f32 psum pair on DVE (no ACT). The
    per-partition bias s_hat applies once via a fused STT; the last pair
    is path B after a pre-fold so the critical tail stays short.
  - Partition-direction hn min via PE transposes + DVE free-dim reduces;
    sqrt/relu/mean and the final combine on host.
"""

import sys

import numpy as np

if "/opt/trn_rl_repo" not in sys.path:
    sys.path.insert(0, "/opt/trn_rl_repo")

from concourse import bacc, bass, mybir, tile
from concourse.bass_utils import run_bass_kernel_spmd

B = 8192
D = 128
C = 128
N_CORES = 8
R = B // N_CORES          # anchors per core
NCH = B // 128            # column chunks (64)
RT = R // 128             # anchor blocks for the tail transposes (8)
PEN = 1024.0

F16 = mybir.dt.float16
F32 = mybir.dt.float32
ALU = mybir.AluOpType
ACTF = mybir.ActivationFunctionType

_NC_CACHE = {}


def _build_nc(kown):
    npair = (NCH - kown) // 2
    nown = kown // 2
    bset = {3, 8, 13, 17, 21, npair - 2, npair - 1}
    nc = bacc.Bacc(None, target_bir_lowering=False)

    xt_d = nc.declare_dram_parameter("xt", [128, B], F16, isOutput=False)
    xls_d = nc.declare_dram_parameter("xls", [128, R], F16, isOutput=False)
    fdr_d = nc.declare_dram_parameter("fdr", [128, R], F16, isOutput=True)
    hop_d = nc.declare_dram_parameter("hop", [128, nown * 2048], F16, isOutput=True)

    with tile.TileContext(nc) as tc:
        with tc.tile_pool(name="const", bufs=1) as cp:
            XTS = [cp.tile([128, 1024], F16, name=f"xts{s}") for s in range(8)]
            XLS = cp.tile([128, R], F16)

            # first dealt pair needs only XLS + one quarter-slice of XT
            nc.scalar.dma_start(XLS[:, 0:512], xls_d[:, 0:512])
            nc.scalar.dma_start(XLS[:, 512:1024], xls_d[:, 512:1024])
            nc.sync.dma_start(XTS[1][:, 256:512], xt_d[:, 1280:1536])
            nc.sync.dma_start(XTS[1][:, 0:256], xt_d[:, 1024:1280])
            nc.sync.dma_start(XTS[1][:, 512:1024], xt_d[:, 1536:2048])
            nc.sync.dma_start(XTS[0][:], xt_d[:, 0:1024])
            nc.sync.dma_start(XTS[2][:], xt_d[:, 2048:3072])
            for s in range(3, 8):
                nc.sync.dma_start(XTS[s][:], xt_d[:, s * 1024 : (s + 1) * 1024])

            def chunk_lhs(ch):
                return XTS[ch // 8][:, (ch % 8) * 128 : (ch % 8) * 128 + 128]

            # spread the own pairs so their host-ship DMAs don't cluster
            sched = []
            dq = [("d", t) for t in range(npair - 1)]
            oq = [("o", u) for u in range(nown)]
            di = 0
            for i in range(len(dq) + len(oq)):
                if oq and di >= 2 and (di - 2) % 5 == 0 and i > 0 and sched[-1][0] == "d":
                    sched.append(oq.pop(0))
                elif dq:
                    sched.append(dq.pop(0))
                    di += 1
                else:
                    sched.append(oq.pop(0))

            # wide tree: every entry is a [128, 2048] tile; merges are wide
            # two-tile TTs; the single [2048]->[1024] fold happens at the end
            levels = {}

            def tree_add(level, ent, tp):
                buf = levels.get(level)
                if buf is None:
                    levels[level] = ent
                else:
                    levels[level] = None
                    nb = tp.tile([128, 2048], F16, tag=f"wt{level}")
                    nc.vector.tensor_tensor(nb[:], buf, ent, op=ALU.min)
                    tree_add(level + 1, nb[:], tp)

            wb_state = [None]

            def add_w(write_fn, tp):
                if wb_state[0] is None:
                    nb = tp.tile([128, 2048], F16, tag="wb")
                    write_fn(nb[:, 0:1024])
                    wb_state[0] = nb
                else:
                    buf = wb_state[0]
                    wb_state[0] = None
                    write_fn(buf[:, 1024:2048])
                    tree_add(0, buf[:], tp)

            with (
                tc.tile_pool(name="dpsum", bufs=2, space=bass.MemorySpace.PSUM) as pd,
                tc.tile_pool(name="opool", bufs=3) as op,
                tc.tile_pool(name="gpool", bufs=4) as gp,
                tc.tile_pool(name="tpool", bufs=2) as tp,
            ):
                def emit_mms(ps, kind, t):
                    for j in range(2):
                        lhs = chunk_lhs((2 * t + j) if kind == "o"
                                        else kown + 2 * t + j)
                        for h in range(2):
                            sl = slice(j * 1024 + h * 512, j * 1024 + (h + 1) * 512)
                            nc.tensor.matmul(ps[:, sl], lhs,
                                             XLS[:, h * 512 : (h + 1) * 512],
                                             start=True, stop=True)

                for kind, t in sched:
                    ps = pd.tile([128, 2048], F32, tag="ps")
                    emit_mms(ps, kind, t)
                    if kind == "o":
                        T2 = op.tile([128, 2048], F16, tag="t2")
                        nc.scalar.activation(T2[:], ps[:], ACTF.Copy)
                        nc.sync.dma_start(
                            hop_d[:, t * 2048 : (t + 1) * 2048], T2[:])
                    elif t in bset:
                        add_w(lambda dst, p=ps: nc.vector.tensor_reduce(
                            dst, p[:].rearrange("p (c i) -> p i c", c=2),
                            axis=mybir.AxisListType.X, op=ALU.min), tp)
                    else:
                        G = gp.tile([128, 2048], F16, tag="g")
                        nc.scalar.activation(G[:], ps[:], ACTF.Copy)
                        tree_add(0, G[:], tp)

                pend = [levels[lv] for lv in sorted(levels)
                        if levels[lv] is not None]
                nx = 0
                while len(pend) > 1:
                    a = pend.pop(0)
                    b = pend.pop(0)
                    nb = tp.tile([128, 2048], F16, tag=f"wx{nx}")
                    nx += 1
                    nc.vector.tensor_tensor(nb[:], a, b, op=ALU.min)
                    pend.append(nb[:])
                PRE = tp.tile([128, R], F16, tag="pre")
                nc.vector.tensor_tensor(PRE[:], pend[0][:, 0:1024],
                                        pend[0][:, 1024:2048], op=ALU.min)
                if wb_state[0] is not None:
                    # lone strided W: only its first half is valid
                    nc.vector.tensor_tensor(PRE[:], PRE[:],
                                            wb_state[0][:, 0:1024], op=ALU.min)
                # last dealt pair: strided reduce merged post-collapse; the
                # s_hat bias and the partition-direction min happen on host
                ps = pd.tile([128, 2048], F32, tag="ps")
                emit_mms(ps, "d", npair - 1)
                WL = tp.tile([128, R], F16, tag="wl")
                nc.vector.tensor_reduce(
                    WL[:], ps[:].rearrange("p (c i) -> p i c", c=2),
                    axis=mybir.AxisListType.X, op=ALU.min)
                FD = tp.tile([128, R], F16, tag="fd")
                nc.vector.tensor_tensor(FD[:], PRE[:], WL[:], op=ALU.min)
                nc.sync.dma_start(fdr_d[:], FD[:])

    nc.compile()
    return nc


def _get_nc(kown):
    if kown not in _NC_CACHE:
        _NC_CACHE[kown] = _build_nc(kown)
    return _NC_CACHE[kown]


def _prep_in_maps(embeddings, labels):
    x = np.asarray(embeddings, dtype=np.float32)
    lab = np.asarray(labels).astype(np.int64)
    order = np.argsort(lab, kind="stable")
    lab_s = lab[order]
    xs = x[order]
    xt = np.ascontiguousarray(xs.T).astype(np.float16)   # [128, B]
    sq = (xs.astype(np.float64) ** 2).sum(1).astype(np.float32)
    idn = np.eye(128, dtype=np.float16)
    own_sets = []
    K = 0
    for m in range(N_CORES):
        mylab = lab_s[m * R : (m + 1) * R]
        own_idx = np.flatnonzero((lab_s >= mylab.min()) & (lab_s <= mylab.max()))
        own_sets.append(own_idx)
        K = max(K, -(-len(own_idx) // 128))
    K += K % 2
    in_maps = []
    extras = []
    for m in range(N_CORES):
        own_idx = own_sets[m]
        mask = np.zeros(B, bool)
        mask[own_idx] = True
        non_own = np.flatnonzero(~mask)
        n_fill = K * 128 - len(own_idx)
        fill, dealt = non_own[:n_fill], non_own[n_fill:]
        own_cols = np.concatenate([own_idx, fill])
        dsort = dealt[np.argsort(sq[dealt], kind="stable")]
        deal_mat = dsort.reshape(128, NCH - K)           # [partition, chunk]
        cols = np.concatenate([own_cols, deal_mat.T.reshape(-1)])
        in_maps.append({
            "xt": np.ascontiguousarray(xt[:, cols]),
            "xls": np.ascontiguousarray(
                (-2.0 * xs[m * R : (m + 1) * R].T)).astype(np.float16),
        })
        shat = sq[deal_mat].mean(1, dtype=np.float64).astype(np.float32)
        extras.append((own_cols, shat))
    return in_maps, lab, order, lab_s, sq, K, extras


def run_cores(embeddings, labels, trace=False, **kw):
    in_maps, lab, order, lab_s, sq, K, extras = _prep_in_maps(embeddings, labels)
    nc = _get_nc(K)
    res = run_bass_kernel_spmd(nc, in_maps, list(range(N_CORES)), trace=trace, **kw)
    hn2_parts, hp2_parts = [], []
    for m, r in enumerate(res.results):
        own_cols, shat = extras[m]
        hn2 = (np.asarray(r["fdr"], np.float32) + shat[:, None]).min(0)
        V = np.asarray(r["hop"], np.float32).reshape(128, K // 2, 2, R)
        V = V.transpose(1, 2, 0, 3).reshape(K * 128, R)
        D2 = V + sq[own_cols][:, None]
        mylab = lab_s[m * R : (m + 1) * R]
        same = lab_s[own_cols][:, None] == mylab[None, :]
        selfm = own_cols[:, None] == (m * R + np.arange(R))[None, :]
        hn2 = np.minimum(hn2, np.where(~same, D2, np.inf).min(0))
        hp2 = np.where(same & ~selfm, D2, -np.inf).max(0)
        hn2_parts.append(hn2)
        hp2_parts.append(hp2)
    hn2 = np.concatenate(hn2_parts)
    hp2 = np.concatenate(hp2_parts)
    hn = np.sqrt(np.maximum(hn2 + sq, 0.0))
    hp = np.sqrt(np.maximum(np.where(np.isfinite(hp2), hp2, -sq) + sq, 0.0))
    pr_sorted = np.maximum(hp - hn + 1.0, 0.0)
    pr = np.empty(B, np.float32)
    pr[order] = pr_sorted
    counts = np.bincount(lab, minlength=C)
    valid = (counts[lab] >= 2) & (counts[lab] <= B - 1)
    nv = int(valid.sum())
    loss = float((pr * valid).sum() / nv) if nv > 0 else 0.0
    return np.float32(loss), res


def kernel(embeddings, labels):
    loss, _ = run_cores(embeddings, labels, trace=False)
    return loss


# revision 38
# speedup vs baseline: 1.0354x; 1.0354x over previous
"""BatchHardTripletLoss on 8 Trainium2 NeuronCores — flipped + norm-dealt.

Layout: rows label-sorted on host; each core streams its 1024 anchors
(free dim) against all B=8192 embeddings as 64 column chunks of 128
(partition dim), processed as 32 psum pairs [128, 2048].

  - Own pairs (K/2 ~ 5): chunks holding the core's own labels (plus
    fillers), spaced through the schedule. Raw -2x.x tiles are ACT-copied
    to f16 and shipped to the HOST over the idle DMA path; the host adds
    exact sq_j, masks same-label/self pairs, and computes hardest-pos
    plus the own-side hardest-neg. No penalty matmuls, no own DVE work.
  - Dealt pairs: remaining columns norm-sorted and dealt so sq_j is
    near-constant per partition. Two paths balanced across engines:
    (A) ACT Copy + one wide [2048] f16 chain min on DVE; (B) one direct
    chain min against the f32 psum pair on DVE (no ACT). The
    per-partition bias s_hat applies once via a fused STT; the last pair
    is path B after a pre-fold so the critical tail stays short.
  - Partition-direction hn min via PE transposes + DVE free-dim reduces;
    sqrt/relu/mean and the final combine on host.
"""

import sys

import numpy as np

if "/opt/trn_rl_repo" not in sys.path:
    sys.path.insert(0, "/opt/trn_rl_repo")

from concourse import bacc, bass, mybir, tile
from concourse.bass_utils import run_bass_kernel_spmd

B = 8192
D = 128
C = 128
N_CORES = 8
R = B // N_CORES          # anchors per core
NCH = B // 128            # column chunks (64)
RT = R // 128             # anchor blocks for the tail transposes (8)
PEN = 1024.0

F16 = mybir.dt.float16
F32 = mybir.dt.float32
ALU = mybir.AluOpType
ACTF = mybir.ActivationFunctionType

_NC_CACHE = {}


def _build_nc(kown):
    npair = (NCH - kown) // 2
    nown = kown // 2
    bset = {3, 6, 10, 13, 17, 20, 24, npair - 1}
    nc = bacc.Bacc(None, target_bir_lowering=False)

    xt_d = nc.declare_dram_parameter("xt", [128, B], F16, isOutput=False)
    xls_d = nc.declare_dram_parameter("xls", [128, R], F16, isOutput=False)
    fdr_d = nc.declare_dram_parameter("fdr", [128, R], F16, isOutput=True)
    hop_d = nc.declare_dram_parameter("hop", [128, nown * 2048], F16, isOutput=True)

    with tile.TileContext(nc) as tc:
        with tc.tile_pool(name="const", bufs=1) as cp:
            XTS = [cp.tile([128, 1024], F16, name=f"xts{s}") for s in range(8)]
            XLS = cp.tile([128, R], F16)

            # first dealt pair needs only XLS + one quarter-slice of XT
            nc.scalar.dma_start(XLS[:, 0:512], xls_d[:, 0:512])
            nc.scalar.dma_start(XLS[:, 512:1024], xls_d[:, 512:1024])
            nc.sync.dma_start(XTS[1][:, 256:512], xt_d[:, 1280:1536])
            nc.sync.dma_start(XTS[1][:, 0:256], xt_d[:, 1024:1280])
            nc.sync.dma_start(XTS[1][:, 512:1024], xt_d[:, 1536:2048])
            nc.sync.dma_start(XTS[0][:], xt_d[:, 0:1024])
            nc.sync.dma_start(XTS[2][:], xt_d[:, 2048:3072])
            for s in range(3, 8):
                nc.sync.dma_start(XTS[s][:], xt_d[:, s * 1024 : (s + 1) * 1024])

            def chunk_lhs(ch):
                return XTS[ch // 8][:, (ch % 8) * 128 : (ch % 8) * 128 + 128]

            # spread the own pairs so their host-ship DMAs don't cluster
            sched = []
            dq = [("d", t) for t in range(npair - 1)]
            oq = [("o", u) for u in range(nown)]
            di = 0
            for i in range(len(dq) + len(oq)):
                if oq and di >= 2 and (di - 2) % 5 == 0 and i > 0 and sched[-1][0] == "d":
                    sched.append(oq.pop(0))
                elif dq:
                    sched.append(dq.pop(0))
                    di += 1
                else:
                    sched.append(oq.pop(0))

            levels = {}

            def tree_push(level, write_fn, tp):
                buf = levels.get(level)
                if buf is None:
                    nb = tp.tile([128, 2048], F16, tag=f"tr{level}")
                    write_fn(nb[:, 0:1024])
                    levels[level] = nb
                else:
                    write_fn(buf[:, 1024:2048])
                    levels[level] = None
                    tree_push(level + 1,
                              lambda dst, b=buf: nc.vector.tensor_tensor(
                                  dst, b[:, 0:1024], b[:, 1024:2048], op=ALU.min),
                              tp)

            with (
                tc.tile_pool(name="dpsum", bufs=2, space=bass.MemorySpace.PSUM) as pd,
                tc.tile_pool(name="opool", bufs=3) as op,
                tc.tile_pool(name="gpool", bufs=4) as gp,
                tc.tile_pool(name="tpool", bufs=2) as tp,
            ):
                def emit_mms(ps, kind, t):
                    for j in range(2):
                        lhs = chunk_lhs((2 * t + j) if kind == "o"
                                        else kown + 2 * t + j)
                        for h in range(2):
                            sl = slice(j * 1024 + h * 512, j * 1024 + (h + 1) * 512)
                            nc.tensor.matmul(ps[:, sl], lhs,
                                             XLS[:, h * 512 : (h + 1) * 512],
                                             start=True, stop=True)

                for kind, t in sched:
                    ps = pd.tile([128, 2048], F32, tag="ps")
                    emit_mms(ps, kind, t)
                    if kind == "o":
                        T2 = op.tile([128, 2048], F16, tag="t2")
                        nc.scalar.activation(T2[:], ps[:], ACTF.Copy)
                        nc.sync.dma_start(
                            hop_d[:, t * 2048 : (t + 1) * 2048], T2[:])
                    elif t in bset:
                        tree_push(0, lambda dst, p=ps: nc.vector.tensor_reduce(
                            dst, p[:].rearrange("p (c i) -> p i c", c=2),
                            axis=mybir.AxisListType.X, op=ALU.min), tp)
                    else:
                        G = gp.tile([128, 2048], F16, tag="g")
                        nc.scalar.activation(G[:], ps[:], ACTF.Copy)
                        tree_push(0, lambda dst, g=G: nc.vector.tensor_tensor(
                            dst, g[:, 0:1024], g[:, 1024:2048], op=ALU.min), tp)

                pend = [levels[lv][:, 0:1024]
                        for lv in sorted(levels) if levels[lv] is not None]
                nx = 0
                while len(pend) > 1:
                    a = pend.pop(0)
                    b = pend.pop(0)
                    nb = tp.tile([128, R], F16, tag=f"trx{nx}")
                    nx += 1
                    nc.vector.tensor_tensor(nb[:], a, b, op=ALU.min)
                    pend.append(nb[:])
                PRE = pend[0]
                # last dealt pair: strided reduce merged post-collapse; the
                # s_hat bias and the partition-direction min happen on host
                ps = pd.tile([128, 2048], F32, tag="ps")
                emit_mms(ps, "d", npair - 1)
                WL = tp.tile([128, R], F16, tag="wl")
                nc.vector.tensor_reduce(
                    WL[:], ps[:].rearrange("p (c i) -> p i c", c=2),
                    axis=mybir.AxisListType.X, op=ALU.min)
                FD = tp.tile([128, R], F16, tag="fd")
                nc.vector.tensor_tensor(FD[:], PRE, WL[:], op=ALU.min)
                nc.sync.dma_start(fdr_d[:], FD[:])

    nc.compile()
    return nc


def _get_nc(kown):
    if kown not in _NC_CACHE:
        _NC_CACHE[kown] = _build_nc(kown)
    return _NC_CACHE[kown]


def _prep_in_maps(embeddings, labels):
    x = np.asarray(embeddings, dtype=np.float32)
    lab = np.asarray(labels).astype(np.int64)
    order = np.argsort(lab, kind="stable")
    lab_s = lab[order]
    xs = x[order]
    xt = np.ascontiguousarray(xs.T).astype(np.float16)   # [128, B]
    sq = (xs.astype(np.float64) ** 2).sum(1).astype(np.float32)
    idn = np.eye(128, dtype=np.float16)
    own_sets = []
    K = 0
    for m in range(N_CORES):
        mylab = lab_s[m * R : (m + 1) * R]
        own_idx = np.flatnonzero((lab_s >= mylab.min()) & (lab_s <= mylab.max()))
        own_sets.append(own_idx)
        K = max(K, -(-len(own_idx) // 128))
    K += K % 2
    in_maps = []
    extras = []
    for m in range(N_CORES):
        own_idx = own_sets[m]
        mask = np.zeros(B, bool)
        mask[own_idx] = True
        non_own = np.flatnonzero(~mask)
        n_fill = K * 128 - len(own_idx)
        fill, dealt = non_own[:n_fill], non_own[n_fill:]
        own_cols = np.concatenate([own_idx, fill])
        dsort = dealt[np.argsort(sq[dealt], kind="stable")]
        deal_mat = dsort.reshape(128, NCH - K)           # [partition, chunk]
        cols = np.concatenate([own_cols, deal_mat.T.reshape(-1)])
        in_maps.append({
            "xt": np.ascontiguousarray(xt[:, cols]),
            "xls": np.ascontiguousarray(
                (-2.0 * xs[m * R : (m + 1) * R].T)).astype(np.float16),
        })
        shat = sq[deal_mat].mean(1, dtype=np.float64).astype(np.float32)
        extras.append((own_cols, shat))
    return in_maps, lab, order, lab_s, sq, K, extras


def run_cores(embeddings, labels, trace=False, **kw):
    in_maps, lab, order, lab_s, sq, K, extras = _prep_in_maps(embeddings, labels)
    nc = _get_nc(K)
    res = run_bass_kernel_spmd(nc, in_maps, list(range(N_CORES)), trace=trace, **kw)
    hn2_parts, hp2_parts = [], []
    for m, r in enumerate(res.results):
        own_cols, shat = extras[m]
        hn2 = (np.asarray(r["fdr"], np.float32) + shat[:, None]).min(0)
        V = np.asarray(r["hop"], np.float32).reshape(128, K // 2, 2, R)
        V = V.transpose(1, 2, 0, 3).reshape(K * 128, R)
        D2 = V + sq[own_cols][:, None]
        mylab = lab_s[m * R : (m + 1) * R]
        same = lab_s[own_cols][:, None] == mylab[None, :]
        selfm = own_cols[:, None] == (m * R + np.arange(R))[None, :]
        hn2 = np.minimum(hn2, np.where(~same, D2, np.inf).min(0))
        hp2 = np.where(same & ~selfm, D2, -np.inf).max(0)
        hn2_parts.append(hn2)
        hp2_parts.append(hp2)
    hn2 = np.concatenate(hn2_parts)
    hp2 = np.concatenate(hp2_parts)
    hn = np.sqrt(np.maximum(hn2 + sq, 0.0))
    hp = np.sqrt(np.maximum(np.where(np.isfinite(hp2), hp2, -sq) + sq, 0.0))
    pr_sorted = np.maximum(hp - hn + 1.0, 0.0)
    pr = np.empty(B, np.float32)
    pr[order] = pr_sorted
    counts = np.bincount(lab, minlength=C)
    valid = (counts[lab] >= 2) & (counts[lab] <= B - 1)
    nv = int(valid.sum())
    loss = float((pr * valid).sum() / nv) if nv > 0 else 0.0
    return np.float32(loss), res


def kernel(embeddings, labels):
    loss, _ = run_cores(embeddings, labels, trace=False)
    return loss
